# revision 48
# baseline (speedup 1.0000x reference)
"""Trainium2 Bass kernel for nn_CNNPolicyHead (KataGo-style CNN policy head).

Contract: kernel(**inputs) takes FULL unsharded inputs (as produced by the
reference setup_inputs) and returns the FULL output [1024, 6, 362] fp32.

Strategy: pure data parallel over 8 NeuronCores -- batch N=1024 sharded 128
per core; all params replicated.

Key structure: ITEM PAIRS. Matmul outputs may only land at 32-aligned PSUM
partition bases, and a 48-channel block fits at bases {0, 64}. So each PSUM
tile holds TWO items' channel blocks (A at rows 0:48, B at rows 64:112,
rows 48:64 forced to zero by 64-wide zero-padded stationaries). Every
ACT/DVE elementwise or reduce op then processes two items per instruction
(112 of 128 lanes), halving the per-item instruction count of the
activation/pooling path -- which is the HW bottleneck (per-instruction
overheads dominate over raw lane throughput).

Dataflow per core, per pair (iA=2j, iB=2j+1), group = 2 pairs:
  x bf16 [128, npc, 3, 362] host-repacked (partition p holds channels
  {p, 128+p, 256+p}; cols padded 361->362, zeros). Chunk loads split
  across the SP HWDGE queue and the Pool SWDGE queue.
  PE: 3+3 matmuls -> psum_g [112,362] (A-g rows 0:64 incl zero pad, B-g
      rows 64:112); 3+3 matmuls -> psum_p likewise. bf16 weights.
  ACT: outg = relu(psum_g + betag_pair), accum -> gsum strip col (fused)
  DVE: gmaxraw col = rowmax(psum_g)  (relu(max+b) == max(relu(+b)))
  per group (2 pairs): small DVE ops build Gmean/Gmoff/Gmax pair-stacked
      strips [112, npc/2]; block-diagonal [112,112] stationaries compute
      bias_pair and passrelu_pair for both items of both pairs at once.
  ACT/DVE (alternating): outp_pair [112,362] bf16 = relu(psum_p + bias col)
  PE: one [112,4] block-diag matmul -> psum2 [4,362] = both items' 2-channel
      conv2 output; a 1-col matmul overwrites pad col 361 with pass logits.
  copy psum2 -> stage [4, pairs, 362] (ACT/DVE alternating), one DMA per
      outevery groups to DRAM out [2, 2, npc/2, 362] bf16 (partition dim
      (item-in-pair, ch) native; host unshards + upcasts to fp32).

mask is all-ones by construction (spec fill=ones); mask_sum_hw is consumed
as data via host-prepped per-item scalars (inv_ms, offinv).
"""
import sys

if "/opt/trn_rl_repo" not in sys.path:
    sys.path.insert(0, "/opt/trn_rl_repo")

import numpy as np

N, C_IN, HW = 1024, 384, 361
HWP = 362
C_P1, C_G1 = 48, 48
N_CORES = 8
NPC = N // N_CORES   # items per core
NPAIR = NPC // 2
GROUP = 4            # items per group (2 pairs)
XBUFS = 4

_cache = {}


def _build(npc=NPC, group=GROUP, xbufs=XBUFS, gbufs=4, pbufs=4,
           stbufs=3, psgb=2, pspb=3, ps2b=2, repeat=1, ablate=None,
           ldchunk=None, qsplit="sp_pool", qout="pool", alt_outp=True,
           outevery=2, stage_alt=True):
    import concourse.bacc as bacc
    import concourse.mybir as mybir
    import concourse.tile as tile

    f32 = mybir.dt.float32
    bf16 = mybir.dt.bfloat16
    AF = mybir.ActivationFunctionType
    ALU = mybir.AluOpType
    AX = mybir.AxisListType

    assert group % 2 == 0
    if ldchunk is None:
        ldchunk = group
    assert ldchunk % group == 0 and npc % ldchunk == 0
    assert (npc // group) % outevery == 0
    half = ldchunk // 2
    npcp = npc // 2
    nc = bacc.Bacc("TRN2", target_bir_lowering=False, debug=False)

    x_d = nc.dram_tensor("x", [128, npc, 3, HWP], bf16, kind="ExternalInput")
    w1g_d = nc.dram_tensor("w1g", [128, 3, 64], bf16, kind="ExternalInput")
    w1p_d = nc.dram_tensor("w1p", [128, 3, 64], bf16, kind="ExternalInput")
    w2pr_d = nc.dram_tensor("w2pr", [112, 4], bf16, kind="ExternalInput")
    wlgp_d = nc.dram_tensor("wlgp", [112, 3, 112], f32, kind="ExternalInput")
    wpp_d = nc.dram_tensor("wpp", [112, 3, 112], f32, kind="ExternalInput")
    wp2p_d = nc.dram_tensor("wp2p", [112, 4], f32, kind="ExternalInput")
    betagp_d = nc.dram_tensor("betagp", [112, 1], f32, kind="ExternalInput")
    beta2p_d = nc.dram_tensor("beta2p", [112, 1], f32, kind="ExternalInput")
    bpassp_d = nc.dram_tensor("bpassp", [112, 1], f32, kind="ExternalInput")
    invmsp_d = nc.dram_tensor("invmsp", [112, npcp], f32,
                              kind="ExternalInput")
    offinvp_d = nc.dram_tensor("offinvp", [112, npcp], f32,
                               kind="ExternalInput")
    # [item-in-pair, ch, pair, l] bf16: partition dim (i c) is native
    out_d = nc.dram_tensor("out", [2, 2, npc // 2, HWP], bf16,
                           kind="ExternalOutput")

    with tile.TileContext(nc) as tc:
        with (
            tc.tile_pool(name="const", bufs=1) as cpool,
            tc.tile_pool(name="x", bufs=xbufs) as xpool,
            tc.tile_pool(name="outg", bufs=gbufs) as gpool,
            tc.tile_pool(name="outp", bufs=pbufs) as ppool,
            tc.tile_pool(name="small", bufs=6) as spool,
            tc.tile_pool(name="grp", bufs=3) as bgpool,
            tc.tile_pool(name="stage", bufs=stbufs) as stpool,
            tc.tile_pool(name="psg", bufs=psgb, space="PSUM") as psgp,
            tc.tile_pool(name="psp", bufs=pspb, space="PSUM") as pspp,
            tc.tile_pool(name="ps2", bufs=ps2b, space="PSUM") as ps2p,
            tc.tile_pool(name="pssm", bufs=1, space="PSUM") as pssm,
        ):
            w1g_sb = cpool.tile([128, 3, 64], bf16)
            w1p_sb = cpool.tile([128, 3, 64], bf16)
            w2pr_sb = cpool.tile([112, 4], bf16)
            wlgp_sb = cpool.tile([112, 3, 112], f32)
            wpp_sb = cpool.tile([112, 3, 112], f32)
            wp2p_sb = cpool.tile([112, 4], f32)
            betagp_sb = cpool.tile([112, 1], f32)
            beta2p_sb = cpool.tile([112, 1], f32)
            bpassp_sb = cpool.tile([112, 1], f32)
            invmsp_sb = cpool.tile([112, npcp], f32)
            offinvp_sb = cpool.tile([112, npcp], f32)
            gsum_s = cpool.tile([112, npcp], f32)
            Gmean_s = cpool.tile([112, npcp], f32)
            Gmoff_s = cpool.tile([112, npcp], f32)
            Gmax_s = cpool.tile([112, npcp], f32)

            for sb, d in [
                (w1g_sb, w1g_d), (w1p_sb, w1p_d), (w2pr_sb, w2pr_d),
                (wlgp_sb, wlgp_d), (wpp_sb, wpp_d), (wp2p_sb, wp2p_d),
                (betagp_sb, betagp_d), (beta2p_sb, beta2p_d),
                (bpassp_sb, bpassp_d), (invmsp_sb, invmsp_d),
                (offinvp_sb, offinvp_d),
            ]:
                nc.sync.dma_start(sb[:], d.ap()[:])

            if ablate == "peonly":
                x_pe = cpool.tile([128, ldchunk, 3, HWP], bf16)
                nc.sync.dma_start(x_pe[:], x_d.ap()[:, 0:ldchunk])

            gidx = 0
            for lc in [cc for _ in range(repeat) for cc in range(npc // ldchunk)]:
              l0 = lc * ldchunk
              if ablate == "peonly":
                  x_r = x_pe
              else:
                x_r = xpool.tile([128, ldchunk, 3, HWP], bf16, tag="x")
                nc.sync.dma_start(
                    x_r[:, 0:half], x_d.ap()[:, l0:l0 + half]
                )
                q2 = nc.scalar if qsplit == "sp_act" else nc.gpsimd
                q2.dma_start(
                    x_r[:, half:ldchunk],
                    x_d.ap()[:, l0 + half:l0 + ldchunk],
                )
              if ablate == "dma":
                  continue
              for g in range(ldchunk // group):
                c0 = l0 + g * group          # first item of group
                pc0 = c0 // 2                # first pair col of group
                gpair = group // 2
                p_tiles = []
                gmaxg = spool.tile([112, gpair], f32, tag="gmax")
                for j2 in range(gpair):
                    pc = pc0 + j2
                    iA = g * group + 2 * j2      # index within chunk
                    iB = iA + 1
                    psg = psgp.tile([112, HWP], f32, tag="psg")
                    for k in range(3):
                        nc.tensor.matmul(
                            psg[0:64, :], w1g_sb[:, k, :], x_r[:, iA, k, :],
                            start=(k == 0), stop=(k == 2),
                        )
                    for k in range(3):
                        nc.tensor.matmul(
                            psg[64:112, :], w1g_sb[:, k, 0:48],
                            x_r[:, iB, k, :],
                            start=(k == 0), stop=(k == 2),
                            skip_group_check=True,
                        )
                    psp = pspp.tile([112, HWP], f32, tag="psp")
                    for k in range(3):
                        nc.tensor.matmul(
                            psp[0:64, :], w1p_sb[:, k, :], x_r[:, iA, k, :],
                            start=(k == 0), stop=(k == 2),
                        )
                    for k in range(3):
                        nc.tensor.matmul(
                            psp[64:112, :], w1p_sb[:, k, 0:48],
                            x_r[:, iB, k, :],
                            start=(k == 0), stop=(k == 2),
                            skip_group_check=True,
                        )
                    p_tiles.append(psp)

                    if ablate == "mm":
                        continue
                    # relu + bias + row-sum for both items at once
                    outg = gpool.tile([112, HW], bf16, tag="outg")
                    nc.scalar.activation(
                        outg[:], psg[:, 0:HW], AF.Relu,
                        bias=betagp_sb[:],
                        accum_out=gsum_s[:, pc:pc + 1],
                    )
                    # rowmax straight from PSUM (no dep on outg)
                    nc.vector.reduce_max(
                        gmaxg[:, j2:j2 + 1], psg[:, 0:HW], axis=AX.X
                    )

                if ablate in ("mm", "peonly"):
                    continue
                pc1 = pc0 + gpair
                # pooled stats, pair-stacked strips
                nc.vector.tensor_scalar(
                    Gmax_s[:, pc0:pc1], gmaxg[:], betagp_sb[:], 0.0,
                    op0=ALU.add, op1=ALU.max,
                )
                nc.vector.scalar_tensor_tensor(
                    Gmean_s[:, pc0:pc1], gsum_s[:, pc0:pc1], 0.0,
                    invmsp_sb[:, pc0:pc1], op0=ALU.add, op1=ALU.mult,
                )
                nc.vector.scalar_tensor_tensor(
                    Gmoff_s[:, pc0:pc1], gsum_s[:, pc0:pc1], 0.0,
                    offinvp_sb[:, pc0:pc1], op0=ALU.add, op1=ALU.mult,
                )
                ps_sm = pssm.tile([112, 4 * gpair], f32, tag="pssm")
                for b, Gs in enumerate((Gmean_s, Gmoff_s, Gmax_s)):
                    nc.tensor.matmul(
                        ps_sm[:, 0:gpair], wlgp_sb[:, b, :],
                        Gs[:, pc0:pc1],
                        start=(b == 0), stop=(b == 2),
                    )
                bias_grp = bgpool.tile([112, gpair], f32, tag="bias")
                nc.vector.tensor_scalar(
                    bias_grp[:], ps_sm[:, 0:gpair], beta2p_sb[:], None,
                    op0=ALU.add,
                )
                for b, Gs in enumerate((Gmean_s, Gmoff_s, Gmax_s)):
                    nc.tensor.matmul(
                        ps_sm[:, gpair:2 * gpair], wpp_sb[:, b, :],
                        Gs[:, pc0:pc1],
                        start=(b == 0), stop=(b == 2),
                        skip_group_check=True,
                    )
                passrelu = bgpool.tile([112, gpair], f32, tag="prelu")
                nc.scalar.activation(
                    passrelu[:], ps_sm[:, gpair:2 * gpair], AF.Relu,
                    bias=bpassp_sb[:],
                )

                if ablate == "nophaseb":
                    continue
                if gidx % outevery == 0:
                    stage = stpool.tile(
                        [4, outevery * gpair, HWP], bf16, tag="stage"
                    )
                    bat0 = c0
                slot = gidx % outevery
                gidx += 1
                for j2 in range(gpair):
                    outp = ppool.tile([112, HWP], bf16, tag="outp")
                    if alt_outp and j2 % 2 == 0:
                        nc.scalar.activation(
                            outp[:], p_tiles[j2][:], AF.Relu,
                            bias=bias_grp[:, j2:j2 + 1],
                        )
                    else:
                        nc.vector.tensor_scalar(
                            outp[:], p_tiles[j2][:],
                            bias_grp[:, j2:j2 + 1], 0.0,
                            op0=ALU.add, op1=ALU.max,
                        )
                    psum2 = ps2p.tile([4, HWP], f32, tag="ps2")
                    nc.tensor.matmul(
                        psum2[:], w2pr_sb[:], outp[:], start=True, stop=True
                    )
                    # pad col 361 <- pass logits for both items of the pair
                    nc.tensor.matmul(
                        psum2[:, HW:HWP], wp2p_sb[:],
                        passrelu[:, j2:j2 + 1],
                        start=True, stop=True, skip_group_check=True,
                    )
                    sslot = slot * gpair + j2
                    if stage_alt and j2 % 2 == 1:
                        nc.scalar.activation(
                            stage[:, sslot, :], psum2[:], AF.Copy
                        )
                    else:
                        nc.vector.tensor_copy(stage[:, sslot, :], psum2[:])
                if slot == outevery - 1:
                    # DRAM view: (c, pairs, item-in-pair, l) ->
                    # partitions (i2 c), free (pairs, l)
                    qo = {"act": nc.scalar, "sp": nc.sync,
                          "pool": nc.gpsimd}[qout]
                    pb0 = bat0 // 2
                    qo.dma_start(
                        out_d.ap()[:, :, pb0:pb0 + outevery * gpair, :]
                        .rearrange("i c p l -> (i c) p l"),
                        stage[:],
                    )

    nc.compile()
    return nc


def _prep_params(inputs):
    """Host-side packing of parameter tensors into pair-stacked layouts."""
    import concourse.mybir as mybir

    bf16 = mybir.dt.np(mybir.dt.bfloat16)
    w_conv1p = np.asarray(inputs["w_conv1p"], np.float32)   # [48, 384]
    w_conv1g = np.asarray(inputs["w_conv1g"], np.float32)

    def conv1_pack(w):
        # [48, 384] -> [128, 3, 64] : [:, k, 0:48] = w[:, k*128:(k+1)*128].T
        out = np.zeros((128, 3, 64), np.float32)
        for k in range(3):
            out[:, k, 0:48] = w[:, k * 128:(k + 1) * 128].T
        return out.astype(bf16)

    w1g = conv1_pack(w_conv1g)
    w1p = conv1_pack(w_conv1p)

    w2t = np.asarray(inputs["w_conv2p"], np.float32).T      # [48, 2]
    w2pr = np.zeros((112, 4), np.float32)
    w2pr[0:48, 0:2] = w2t
    w2pr[64:112, 2:4] = w2t
    w2pr = w2pr.astype(bf16)

    def lin_pack(w):
        # w [48, 144] -> blocks [48,48] per b; pair block-diag [112,3,112]
        wt = np.asarray(w, np.float32).T.reshape(3, 48, 48)  # [b, in48, out48]
        out = np.zeros((112, 3, 112), np.float32)
        for b in range(3):
            out[0:48, b, 0:48] = wt[b]
            out[64:112, b, 64:112] = wt[b]
        return out

    wlgp = lin_pack(inputs["w_linear_g"])
    wpp = lin_pack(inputs["w_linear_pass"])

    wp2t = np.asarray(inputs["w_linear_pass2"], np.float32).T  # [48, 2]
    wp2p = np.zeros((112, 4), np.float32)
    wp2p[0:48, 0:2] = wp2t
    wp2p[64:112, 2:4] = wp2t

    def dup(v):
        out = np.zeros((112, 1), np.float32)
        out[0:48, 0] = v
        out[64:112, 0] = v
        return out

    betagp = dup(np.asarray(inputs["beta_g"], np.float32))
    beta2p = dup(np.asarray(inputs["beta_2"], np.float32))
    bpassp = dup(np.asarray(inputs["b_linear_pass"], np.float32))

    ms = np.asarray(inputs["mask_sum_hw"], np.float32).reshape(-1)  # [N]
    invms = (1.0 / ms).astype(np.float32)
    offinv = (((np.sqrt(ms) - 14.0) / 10.0) / ms).astype(np.float32)
    return dict(
        w1g=w1g, w1p=w1p, w2pr=w2pr, wlgp=wlgp, wpp=wpp, wp2p=wp2p,
        betagp=betagp, beta2p=beta2p, bpassp=bpassp,
    ), invms, offinv


def _pair_scalars(v):
    """[npc] per-item -> [112, npc/2] pair-stacked (A rows 0:48, B 64:112)."""
    npc = v.shape[0]
    out = np.zeros((112, npc // 2), np.float32)
    out[0:48, :] = v[0::2][None, :]
    out[64:112, :] = v[1::2][None, :]
    return out


def _prep_x(x_full):
    """Repack x [N, 384, 361] -> per-core [128, NPC, 3, 362] bf16 (p-major,
    padded). Channel c = k*128 + p lands at partition p, row k; pad col 361
    is zero. The bf16 downcast (~0.3% rel) halves the HBM x traffic, which
    is the kernel's roofline."""
    import concourse.mybir as mybir

    bf16 = mybir.dt.np(mybir.dt.bfloat16)
    xk = np.asarray(x_full, np.float32).reshape(N, 3, 128, HW).astype(bf16)
    out = np.zeros((N_CORES, 128, NPC, 3, HWP), bf16)
    for c in range(N_CORES):
        s = slice(c * NPC, (c + 1) * NPC)
        out[c, :, :, :, 0:HW] = xk[s].transpose(2, 0, 1, 3)
    return out


def make_in_maps(inputs):
    params, invms, offinv = _prep_params(inputs)
    xp = _prep_x(np.asarray(inputs["x"], np.float32).reshape(N, C_IN, HW))
    in_maps = []
    for c in range(N_CORES):
        s = slice(c * NPC, (c + 1) * NPC)
        m = dict(params)
        m["x"] = xp[c]
        m["invmsp"] = _pair_scalars(invms[s])
        m["offinvp"] = _pair_scalars(offinv[s])
        in_maps.append(m)
    return in_maps


def kernel(**inputs) -> np.ndarray:
    from concourse import bass_utils

    if "nc" not in _cache:
        _cache["nc"] = _build()
    nc = _cache["nc"]

    in_maps = make_in_maps(inputs)
    res = bass_utils.run_bass_kernel_spmd(
        nc, in_maps, core_ids=list(range(N_CORES))
    )
    _cache["last_result"] = res

    full = np.zeros((N, 6, HW + 1), np.float32)
    for c in range(N_CORES):
        o = np.asarray(res.results[c]["out"], np.float32)  # [2,2,NPC/2,362]
        b = c * NPC
        full[b:b + NPC:2, 0, :] = o[0, 0]
        full[b + 1:b + NPC:2, 0, :] = o[1, 0]
        full[b:b + NPC:2, 5, :] = o[0, 1]
        full[b + 1:b + NPC:2, 5, :] = o[1, 1]
    return full
